# revision 14
# baseline (speedup 1.0000x reference)
"""Trainium2 Bass kernel for a 4-layer spiking network (IF neurons, T=16).

Reference computation (per batch row, fp32):
    c0 = x @ W0.T + b0                      (loop-invariant)
    for t in 1..16:
        v0 += c0;            s0 = (v0>=1); v0 *= (1-s0)
        v1 += s0@W1.T + b1;  s1 = (v1>=1); v1 *= (1-s1)
        v2 += s1@W2.T + b2;  s2 = (v2>=1); v2 *= (1-s2)
        vL = (vL + s2@W3.T + b3) / 2        (NonSpikingLIF, tau=2)
    return vL

v3 design notes (v1 was PE-bound: fp32 matmuls cost 4 cyc/row):
  * The spike dynamics are chaotic: any weight/drive perturbation
    (bf16 0.4%, even tf32 0.02%) explodes the output error, so all
    drives must be exact in fp32.  Exact bf16 split trick: W = hi + lo
    with hi = bf16(W), lo = bf16(W - hi); masks are exactly {0,1} in
    bf16, so mask @ hi + mask @ lo accumulated in fp32 PSUM equals the
    fp32 product to ~2^-18 relative -- at bf16 matmul speed (1 cyc/row,
    2x fewer PE cycles than fp32).  W3 feeds only the linear LIF
    readout (no threshold), plain bf16 there is enough.
  * NOT-spiked mask trick kept: feed m = (v < theta) into the next
    layer with negated weights; constants fold into biases.
  * Layer 0 has constant drive c0, so its spike pattern is periodic:
    with phase psi (steps since last spike, minus 1), m0 = [psi < 1/c0
    - 1] and psi' = (psi + 1)*m0.  c0 is computed once per window by
    an exact fp32 matmul; rec = 1/c0 via the DVE reciprocal (rec
    patched to +BIG where c0 <= 0).  Per step this is one stt per half
    on GPSIMD/DVE -- no per-step c0 add, no layer-0 PE work.
  * Layers 1/2 membranes live in PSUM as single [128,1024] tiles; the
    PE accumulates each step's drive (start=False); masks via one
    1024-wide ACT sigmoid(2^40*(th-u)) -> bf16; reset via one
    1024-wide DVE stt  u <- (u + bhat) * m.
  * LIF readout linear fold kept: PE accumulates m2_t @ (-W3 *
    2^(t-17)).T into one PSUM bank (64/64 partition packing for
    even/odd halves) over all 16 steps; bias added at eviction.
"""

import numpy as np
import ml_dtypes

import concourse.bass as bass
import concourse.bacc as bacc
import concourse.mybir as mybir
from concourse.bass_utils import run_bass_kernel_spmd
from concourse.tile import TileContext

F32 = mybir.dt.float32
BF16 = mybir.dt.bfloat16

B = 65536
IN = 128
H = 128
OUT = 64
T = 16
NCORES = 8
BC = B // NCORES          # batch rows per core (8192)
NB = 512                  # psum bank width in fp32
NB2 = 2 * NB              # window width (1024)
NWIN = BC // NB2          # 8 windows

_CACHE = {}

add = mybir.AluOpType.add
mul = mybir.AluOpType.mult
is_lt = mybir.AluOpType.is_lt
is_le = mybir.AluOpType.is_le
SGM = mybir.ActivationFunctionType.Sigmoid
IDT = mybir.ActivationFunctionType.Identity
NSC = float(-(2.0 ** 40))
BIG = float(2.0 ** 60)


def _build():
    nc = bacc.Bacc("TRN2", debug=False, target_bir_lowering=False,
                   num_swdge_queues=4)

    xT = nc.dram_tensor("xT", [IN, BC], F32, kind="ExternalInput").ap()
    w0T = nc.dram_tensor("w0T", [IN, H], F32, kind="ExternalInput").ap()
    # bf16 consts: w1hi | w1lo | w2hi | w2lo | w3(16 steps)
    WB = 4 * H + T * OUT
    wb = nc.dram_tensor("wb", [H, WB], BF16, kind="ExternalInput").ap()
    # f32 per-partition consts: b0 | sg1 | sg2 | bh1 | bh2 | b3cc
    cf = nc.dram_tensor("cf", [H, 7], F32, kind="ExternalInput").ap()
    outT = nc.dram_tensor("outT", [OUT, BC], F32, kind="ExternalOutput").ap()

    with TileContext(nc) as tc:
        with (
            tc.tile_pool(name="consts", bufs=1) as cpool,
            tc.tile_pool(name="xin", bufs=1) as xpool,
            tc.tile_pool(name="state", bufs=2) as spool,
            tc.tile_pool(name="masks", bufs=3) as mpool,
            tc.tile_pool(name="outs", bufs=2) as opool,
            tc.tile_pool(name="psA", bufs=1, space="PSUM") as pA,
            tc.tile_pool(name="psB", bufs=1, space="PSUM") as pB,
            tc.tile_pool(name="psC", bufs=2, space="PSUM") as pC,
            tc.tile_pool(name="psD", bufs=1, space="PSUM") as pD,
        ):
            wbt = cpool.tile([H, WB], BF16, tag="wbt")
            nc.gpsimd.dma_start(out=wbt[:], in_=wb)
            w1h = wbt[:, 0:H]
            w1l = wbt[:, H:2 * H]
            w2h = wbt[:, 2 * H:3 * H]
            w2l = wbt[:, 3 * H:4 * H]
            w3s = wbt[:, 4 * H:WB]

            cft = cpool.tile([H, 7], F32, tag="cft")
            nc.gpsimd.dma_start(out=cft[:], in_=cf)
            b0c = cft[:, 0:1]
            sg1 = cft[:, 1:2]
            sg2 = cft[:, 2:3]
            bh1 = cft[:, 3:4]
            bh2 = cft[:, 4:5]
            b3c = cft[0:OUT, 5:6]
            b3cO = cft[OUT:2 * OUT, 5:6]
            sg0 = cft[:, 6:7]

            w0t = cpool.tile([IN, H], F32, tag="w0t")
            nc.gpsimd.dma_start(out=w0t[:], in_=w0T)

            xs = xpool.tile([IN, BC], F32, tag="xs")
            nc.gpsimd.dma_start(out=xs[:], in_=xT)

            ones = cpool.tile([H, NB], BF16, tag="ones")
            nc.vector.memset(ones[:], 1.0)

            GRP = 4
            for w in range(NWIN):
                xw = xs[:, w * NB2:(w + 1) * NB2]

                U1 = pA.tile([H, NB2], F32, tag="U1")
                U2 = pB.tile([H, NB2], F32, tag="U2")
                pv = pC.tile([H, NB], F32, tag="pv")

                c0 = spool.tile([H, NB2], F32, tag="c0")
                v0 = spool.tile([H, NB2], F32, tag="v0")
                a0 = spool.tile([H, NB2], F32, tag="a0")

                # ---- per-window: c0 = x@W0.T + b0 (exact fp32 matmul) ----
                nc.tensor.matmul(U1[:, 0:NB], w0t[:], xw[:, 0:NB],
                                 start=True, stop=True, skip_group_check=True)
                nc.tensor.matmul(U1[:, NB:NB2], w0t[:], xw[:, NB:NB2],
                                 start=True, stop=True, skip_group_check=True)
                nc.scalar.activation(c0[:], U1[:], IDT, bias=b0c, scale=1.0)

                for t in range(1, T + 1):
                    st, sp = (t == 1), (t == T)
                    vc = c0 if st else v0

                    m0 = mpool.tile([H, NB2], BF16, tag="m0")
                    m1 = mpool.tile([H, NB2], BF16, tag="m1")
                    m2 = mpool.tile([H, NB2], BF16, tag="m2")

                    # ---- layer 0 (SBUF fp32 membrane) ----
                    # mask via ACT step-sigmoid; reset via self-contained
                    # DVE stt (recomputes the compare); charge on GPSIMD.
                    # The reset/charge feed next step's m0 only, so they
                    # are emitted AFTER this step's U1/U2 resets (below)
                    # to keep the U-recurrence off the DVE queue tail.
                    nc.scalar.activation(m0[:], vc[:], SGM, bias=sg0,
                                         scale=NSC)

                    # ---- layer 1 (PSUM membrane, split-bf16 drive) ----
                    nc.tensor.matmul(U1[:, 0:NB], w1h, m0[:, 0:NB],
                                     start=st, stop=False,
                                     skip_group_check=True)
                    nc.tensor.matmul(U1[:, 0:NB], w1l, m0[:, 0:NB],
                                     start=False, stop=sp,
                                     skip_group_check=True)
                    nc.tensor.matmul(U1[:, NB:NB2], w1h, m0[:, NB:NB2],
                                     start=st, stop=False,
                                     skip_group_check=True)
                    nc.tensor.matmul(U1[:, NB:NB2], w1l, m0[:, NB:NB2],
                                     start=False, stop=sp,
                                     skip_group_check=True)
                    nc.scalar.activation(m1[:], U1[:], SGM, bias=sg1,
                                         scale=NSC)
                    if not sp:
                        nc.vector.scalar_tensor_tensor(
                            U1[:], U1[:], bh1, m1[:], add, mul)

                    # ---- layer 2 ----
                    nc.tensor.matmul(U2[:, 0:NB], w2h, m1[:, 0:NB],
                                     start=st, stop=False,
                                     skip_group_check=True)
                    nc.tensor.matmul(U2[:, 0:NB], w2l, m1[:, 0:NB],
                                     start=False, stop=sp,
                                     skip_group_check=True)
                    nc.tensor.matmul(U2[:, NB:NB2], w2h, m1[:, NB:NB2],
                                     start=st, stop=False,
                                     skip_group_check=True)
                    nc.tensor.matmul(U2[:, NB:NB2], w2l, m1[:, NB:NB2],
                                     start=False, stop=sp,
                                     skip_group_check=True)
                    nc.scalar.activation(m2[:], U2[:], SGM, bias=sg2,
                                         scale=NSC)
                    if not sp:
                        nc.vector.scalar_tensor_tensor(
                            U2[:], U2[:], bh2, m2[:], add, mul)

                    # ---- LIF readout (64/64 partition packing) ----
                    w3t = w3s[:, (t - 1) * OUT:t * OUT]
                    nc.tensor.matmul(pv[0:OUT, :], w3t, m2[:, 0:NB],
                                     start=st, stop=sp, skip_group_check=True)
                    nc.tensor.matmul(pv[OUT:2 * OUT, :], w3t, m2[:, NB:NB2],
                                     start=st, stop=sp, skip_group_check=True)

                    # ---- layer 0 reset+charge (off the critical cycle) ----
                    # v0' = vc*m0 + c0, split: even half on GPSIMD, odd on
                    # DVE (both consume the ACT-produced bf16 mask m0)
                    if not sp:
                        nc.gpsimd.tensor_tensor(
                            a0[:, 0:NB], vc[:, 0:NB], m0[:, 0:NB], mul)
                        nc.gpsimd.tensor_tensor(
                            v0[:, 0:NB], a0[:, 0:NB], c0[:, 0:NB], add)
                        nc.vector.tensor_tensor(
                            a0[:, NB:NB2], vc[:, NB:NB2], m0[:, NB:NB2], mul)
                        nc.vector.tensor_tensor(
                            v0[:, NB:NB2], a0[:, NB:NB2], c0[:, NB:NB2], add)

                    # ---- PE warmers: dependency-free matmuls into the
                    # spare PSUM bank keep the HAM clock gate at 2.4 GHz
                    dmy = pD.tile([H, NB], F32, tag="dmy")
                    nc.tensor.matmul(dmy[:], w1h, ones[:],
                                     start=True, stop=True,
                                     skip_group_check=True)
                    nc.tensor.matmul(dmy[:], w2h, ones[:],
                                     start=True, stop=True,
                                     skip_group_check=True)

                # ---- window output eviction ----
                if w % GRP == 0:
                    ot = opool.tile([OUT, GRP * NB2], F32, tag="ot")
                    _CACHE["_ot"] = ot
                else:
                    ot = _CACHE["_ot"]
                q = (w % GRP) * NB2
                nc.scalar.activation(ot[:, q:q + NB], pv[0:OUT, :], IDT,
                                     bias=b3c, scale=1.0)
                nc.scalar.activation(ot[:, q + NB:q + NB2], pv[OUT:2 * OUT, :],
                                     IDT, bias=b3cO, scale=1.0)
                if w % GRP == GRP - 1 or w == NWIN - 1:
                    g0 = (w // GRP) * GRP
                    wd = (w - g0 + 1) * NB2
                    nc.sync.dma_start(
                        out=outT[:, g0 * NB2:g0 * NB2 + wd], in_=ot[:, 0:wd])

    nc.finalize()
    return nc


def _prep(W0, b0, W1, b1, W2, b2, W3, b3):
    f32 = np.float32
    bf = ml_dtypes.bfloat16
    w0t = np.ascontiguousarray(W0.T).astype(f32)

    def split(wT):  # exact bf16 hi/lo split of -W.T
        neg = np.ascontiguousarray(-wT.T).astype(f32)
        hi = neg.astype(bf)
        lo = (neg - hi.astype(f32)).astype(bf)
        return hi, lo

    w1h, w1l = split(W1)
    w2h, w2l = split(W2)
    w3w = np.concatenate(
        [np.ascontiguousarray(-W3.T) * f32(2.0 ** (t - T - 1))
         for t in range(1, T + 1)], axis=1).astype(bf)
    wb = np.concatenate([w1h, w1l, w2h, w2l, w3w], axis=1)

    # bhat folds the column-sum of the weights as actually used
    b1h = (b1 - (w1h.astype(f32) + w1l.astype(f32)).sum(0)).astype(f32)
    b2h = (b2 - (w2h.astype(f32) + w2l.astype(f32)).sum(0)).astype(f32)
    one = f32(1.0)
    big = f32(2.0 ** 40)
    cf = np.zeros((H, 7), f32)
    cf[:, 0] = b0.astype(f32)
    cf[:, 1] = (one - b1h) * big
    cf[:, 2] = (one - b2h) * big
    cf[:, 3] = b1h
    cf[:, 4] = b2h
    beta3 = (b3 * (1.0 - 2.0 ** (-T))
             - w3w.astype(f32).reshape(H, T, OUT).sum(axis=0).sum(axis=0)
             ).astype(f32)
    cf[:, 6] = big
    cf[:OUT, 5] = beta3
    cf[OUT:2 * OUT, 5] = beta3
    return dict(wb=np.ascontiguousarray(wb), cf=np.ascontiguousarray(cf),
                w0T=np.ascontiguousarray(w0t))


def kernel(x, W0, b0, W1, b1, W2, b2, W3, b3, _trace=False, _trace_kwargs=None):
    if "nc" not in _CACHE:
        _CACHE["nc"] = _build()
    nc = _CACHE["nc"]

    wmap = _prep(W0, b0, W1, b1, W2, b2, W3, b3)
    xTfull = np.ascontiguousarray(x.astype(np.float32).T)   # [IN, B]
    in_maps = []
    for c in range(NCORES):
        m = dict(wmap)
        m["xT"] = np.ascontiguousarray(xTfull[:, c * BC:(c + 1) * BC])
        in_maps.append(m)

    kw = {}
    if _trace:
        kw = dict(trace=True, trace_cores=[0], **(_trace_kwargs or {}))
    res = run_bass_kernel_spmd(nc, in_maps, list(range(NCORES)), **kw)
    out = np.concatenate([r["outT"] for r in res.results], axis=1)  # [OUT, B]
    if _trace:
        _CACHE["last_results"] = res
    return np.ascontiguousarray(out.T)
